# revision 5
# baseline (speedup 1.0000x reference)
"""Trainium2 Bass kernel for a 2-layer SimpleRNN over embedded tokens.

Computation (full shapes): V=50000, D=128, B=512, T=256, U=256
    x = emb[inputs]                                   [B, T, D]
    h0_t = tanh(x_t @ k0 + h0_{t-1} @ rk0 + b0)       [B, U]
    h1_t = tanh(h0_t @ k1 + h1_{t-1} @ rk1 + b1)      [B, U]
    out = sigmoid(h1_{T-1} @ wo + bo)                 [B, 1]

Strategy: data-parallel over batch across 8 cores (64 rows each). All state
kept transposed ([U, batch] layout) so the time-step matmuls keep the full
128-wide stationary dim. Embedding rows are gathered on-device with indirect
DMA, transposed on the PE, and cached in SBUF as bf16. All matmuls run in
bf16 except the precision-critical rk0 @ h0 recurrence, which runs in fp32
(numerically validated: ~1e-3 rel err vs fp32 reference).
"""

import os
import sys

import numpy as np

if "/opt/trn_rl_repo" not in sys.path:
    sys.path.insert(0, "/opt/trn_rl_repo")

import ml_dtypes

import concourse.bacc as bacc
import concourse.bass as bass
import concourse.mybir as mybir
import concourse.tile as tile
from concourse.bass_utils import run_bass_kernel_spmd
from concourse.masks import make_identity

V, D, B, T, U = 50000, 128, 512, 256, 256
NCORES = 8
BS = B // NCORES          # batch rows per core (64)
TOK = BS * T              # tokens per core (16384)
NCHUNK = TOK // 128       # 128-token gather chunks (128)
GATHER_LOOKAHEAD = 6      # chunks emitted ahead of the consuming step

F32 = mybir.dt.float32
BF16 = mybir.dt.bfloat16
I32 = mybir.dt.int32
AF = mybir.ActivationFunctionType


def _build():
    nc = bacc.Bacc(
        "TRN2",
        target_bir_lowering=False,
        debug=False,
        enable_asserts=False,
        num_devices=NCORES,
    )

    emb_d = nc.dram_tensor("emb", [V, D], F32, kind="ExternalInput").ap()
    gidx_d = nc.dram_tensor("gidx", [128, NCHUNK], I32, kind="ExternalInput").ap()
    k0_d = nc.dram_tensor("k0b", [D, U], BF16, kind="ExternalInput").ap()
    rk0_d = nc.dram_tensor("rk0", [U, U], F32, kind="ExternalInput").ap()
    k1_d = nc.dram_tensor("k1b", [U, U], BF16, kind="ExternalInput").ap()
    rk1_d = nc.dram_tensor("rk1b", [U, U], BF16, kind="ExternalInput").ap()
    wo_d = nc.dram_tensor("wot", [128, 2], BF16, kind="ExternalInput").ap()
    b0_d = nc.dram_tensor("b0t", [128, 2], F32, kind="ExternalInput").ap()
    b1_d = nc.dram_tensor("b1t", [128, 2], F32, kind="ExternalInput").ap()
    bo_d = nc.dram_tensor("bot", [1, 1], F32, kind="ExternalInput").ap()
    out_d = nc.dram_tensor("out", [1, BS], F32, kind="ExternalOutput").ap()

    with tile.TileContext(nc) as tc:
        with (
            tc.tile_pool(name="const", bufs=1) as cpool,
            tc.tile_pool(name="xg", bufs=4) as xgpool,
            tc.tile_pool(name="tp", bufs=2, space="PSUM") as tppool,
            tc.tile_pool(name="ps0", bufs=2, space="PSUM") as ps0pool,
            tc.tile_pool(name="ps1", bufs=2, space="PSUM") as ps1pool,
            tc.tile_pool(name="pso", bufs=1, space="PSUM") as psopool,
            tc.tile_pool(name="h0f", bufs=2) as h0fpool,
            tc.tile_pool(name="h0b", bufs=2) as h0bpool,
            tc.tile_pool(name="h1b", bufs=2) as h1bpool,
        ):
            # ---- constants / weights into SBUF ----
            gidx = cpool.tile([128, NCHUNK], I32, name="gidx_sb")
            nc.sync.dma_start(out=gidx[:, :], in_=gidx_d[:, :])
            k0s = cpool.tile([D, U], BF16, name="k0_sb")
            nc.sync.dma_start(out=k0s[:, :], in_=k0_d[:, :])
            rk0s = [cpool.tile([128, U], F32, name=f"rk0_sb{kh}") for kh in (0, 1)]
            k1s = [cpool.tile([128, U], BF16, name=f"k1_sb{kh}") for kh in (0, 1)]
            rk1s = [cpool.tile([128, U], BF16, name=f"rk1_sb{kh}") for kh in (0, 1)]
            for kh in (0, 1):
                sl = slice(kh * 128, (kh + 1) * 128)
                nc.sync.dma_start(out=rk0s[kh][:, :], in_=rk0_d[sl, :])
                nc.sync.dma_start(out=k1s[kh][:, :], in_=k1_d[sl, :])
                nc.sync.dma_start(out=rk1s[kh][:, :], in_=rk1_d[sl, :])
            wos = cpool.tile([128, 2], BF16, name="wo_sb")
            nc.sync.dma_start(out=wos[:, :], in_=wo_d[:, :])
            b0s = cpool.tile([128, 2], F32, name="b0_sb")
            nc.sync.dma_start(out=b0s[:, :], in_=b0_d[:, :])
            b1s = cpool.tile([128, 2], F32, name="b1_sb")
            nc.sync.dma_start(out=b1s[:, :], in_=b1_d[:, :])
            bos = cpool.tile([1, 1], F32, name="bo_sb")
            nc.sync.dma_start(out=bos[:1, :], in_=bo_d[:, :])
            ident = cpool.tile([128, 128], F32, name="ident")
            make_identity(nc, ident[:, :])

            # xT cache: [D, token] bf16, token n = t*BS + b
            xT = cpool.tile([128, TOK], BF16, name="xT")

            def emit_chunk(j):
                """Gather 128 embedding rows -> transpose -> xT[:, j*128:...]."""
                xg = xgpool.tile([128, 128], F32, name="xg", tag="xg")
                nc.gpsimd.indirect_dma_start(
                    out=xg[:, :],
                    out_offset=None,
                    in_=emb_d[:, :],
                    in_offset=bass.IndirectOffsetOnAxis(
                        ap=gidx[:, j : j + 1], axis=0
                    ),
                )
                tp = tppool.tile([128, 128], F32, name="tp", tag="tp")
                nc.tensor.transpose(out=tp[:, :], in_=xg[:, :], identity=ident[:, :])
                nc.vector.tensor_copy(
                    out=xT[:, j * 128 : (j + 1) * 128], in_=tp[:, :]
                )

            def layer1(s, h0b_s, h1b_prev, h1b_pool):
                """Second RNN cell for step s (lagged). Returns h1b_s."""
                ps1 = ps1pool.tile([128, 2 * BS], F32, name="ps1", tag="ps1")
                nmm = 4 if s == 0 else 8
                i = 0
                for kh in (0, 1):
                    rhs = h0b_s[:, kh * BS : (kh + 1) * BS]
                    for mh in (0, 1):
                        nc.tensor.matmul(
                            out=ps1[:, mh * BS : (mh + 1) * BS],
                            lhsT=k1s[kh][:, mh * 128 : (mh + 1) * 128],
                            rhs=rhs,
                            start=(i == 0),
                            stop=(i == nmm - 1),
                        )
                        i += 1
                if s > 0:
                    for kh in (0, 1):
                        rhs = h1b_prev[:, kh * BS : (kh + 1) * BS]
                        for mh in (0, 1):
                            nc.tensor.matmul(
                                out=ps1[:, mh * BS : (mh + 1) * BS],
                                lhsT=rk1s[kh][:, mh * 128 : (mh + 1) * 128],
                                rhs=rhs,
                                start=False,
                                stop=(i == nmm - 1),
                            )
                            i += 1
                h1b = h1b_pool.tile([128, 2 * BS], BF16, name="h1b", tag="h1b")
                for mh in (0, 1):
                    nc.scalar.activation(
                        out=h1b[:, mh * BS : (mh + 1) * BS],
                        in_=ps1[:, mh * BS : (mh + 1) * BS],
                        func=AF.Tanh,
                        bias=b1s[:, mh : mh + 1],
                    )
                return h1b

            # ---- main fused loop; layer 1 lags layer 0 by one step ----
            h0f_prev = h0b_prev = h1b_prev = None
            for t in range(T):
                # prefetch pipeline for the embedding cache
                if t == 0:
                    for j in range(GATHER_LOOKAHEAD):
                        emit_chunk(j)
                elif t % 2 == 0:
                    j = t // 2 + GATHER_LOOKAHEAD - 1
                    if j < NCHUNK:
                        emit_chunk(j)

                xt = xT[:, t * BS : (t + 1) * BS]
                ps0 = ps0pool.tile([128, 2 * BS], F32, name="ps0", tag="ps0")
                nmm = 2 if t == 0 else 6
                i = 0
                for mh in (0, 1):
                    nc.tensor.matmul(
                        out=ps0[:, mh * BS : (mh + 1) * BS],
                        lhsT=k0s[:, mh * 128 : (mh + 1) * 128],
                        rhs=xt,
                        start=(i == 0),
                        stop=(i == nmm - 1),
                    )
                    i += 1
                if t > 0:
                    for kh in (0, 1):
                        rhs = h0f_prev[:, kh * BS : (kh + 1) * BS]
                        for mh in (0, 1):
                            nc.tensor.matmul(
                                out=ps0[:, mh * BS : (mh + 1) * BS],
                                lhsT=rk0s[kh][:, mh * 128 : (mh + 1) * 128],
                                rhs=rhs,
                                start=False,
                                stop=(i == nmm - 1),
                            )
                            i += 1
                h0f = h0fpool.tile([128, 2 * BS], F32, name="h0f", tag="h0f")
                for mh in (0, 1):
                    nc.scalar.activation(
                        out=h0f[:, mh * BS : (mh + 1) * BS],
                        in_=ps0[:, mh * BS : (mh + 1) * BS],
                        func=AF.Tanh,
                        bias=b0s[:, mh : mh + 1],
                    )
                h0b = h0bpool.tile([128, 2 * BS], BF16, name="h0b", tag="h0b")
                nc.vector.tensor_copy(out=h0b[:, :], in_=h0f[:, :])

                if t > 0:
                    h1b_prev = layer1(t - 1, h0b_prev, h1b_prev, h1bpool)
                h0f_prev, h0b_prev = h0f, h0b

            h1b_last = layer1(T - 1, h0b_prev, h1b_prev, h1bpool)

            # ---- output head: sigmoid(h1 @ wo + bo), transposed ----
            pso = psopool.tile([1, BS], F32, name="pso")
            for kh in (0, 1):
                nc.tensor.matmul(
                    out=pso[:1, :],
                    lhsT=wos[:, kh : kh + 1],
                    rhs=h1b_last[:, kh * BS : (kh + 1) * BS],
                    start=(kh == 0),
                    stop=(kh == 1),
                )
            osb = cpool.tile([1, BS], F32, name="osb")
            nc.scalar.activation(
                out=osb[:1, :], in_=pso[:1, :], func=AF.Sigmoid, bias=bos[:1, 0:1]
            )
            nc.sync.dma_start(out=out_d[:, :], in_=osb[:1, :])

    nc.compile()
    return nc


_NC_CACHE = []


def _get_nc():
    if not _NC_CACHE:
        _NC_CACHE.append(_build())
    return _NC_CACHE[0]


def make_in_maps(inputs, emb, k0, rk0, b0, k1, rk1, b1, wo, bo):
    inputs = np.ascontiguousarray(np.asarray(inputs, dtype=np.int32))
    emb = np.ascontiguousarray(np.asarray(emb, dtype=np.float32))
    f32 = lambda a, shp: np.ascontiguousarray(np.asarray(a, np.float32).reshape(shp))
    bf16 = lambda a, shp: np.ascontiguousarray(
        np.asarray(a, np.float32).reshape(shp).astype(ml_dtypes.bfloat16)
    )

    k0b = bf16(k0, (D, U))
    rk0f = f32(rk0, (U, U))
    k1b = bf16(k1, (U, U))
    rk1b = bf16(rk1, (U, U))
    wot = bf16(np.asarray(wo, np.float32).reshape(U).reshape(2, 128).T, (128, 2))
    b0t = f32(np.asarray(b0, np.float32).reshape(2, 128).T, (128, 2))
    b1t = f32(np.asarray(b1, np.float32).reshape(2, 128).T, (128, 2))
    bot = f32(bo, (1, 1))

    in_maps = []
    for c in range(NCORES):
        idx_c = inputs[c * BS : (c + 1) * BS, :]          # [BS, T]
        idx_flat = idx_c.T.reshape(-1)                    # token n = t*BS + b
        gidx = np.ascontiguousarray(idx_flat.reshape(NCHUNK, 128).T.astype(np.int32))
        in_maps.append(
            {
                "emb": emb,
                "gidx": gidx,
                "k0b": k0b,
                "rk0": rk0f,
                "k1b": k1b,
                "rk1b": rk1b,
                "wot": wot,
                "b0t": b0t,
                "b1t": b1t,
                "bot": bot,
            }
        )
    return in_maps


def kernel(inputs, emb, k0, rk0, b0, k1, rk1, b1, wo, bo):
    in_maps = make_in_maps(inputs, emb, k0, rk0, b0, k1, rk1, b1, wo, bo)
    nc = _get_nc()
    res = run_bass_kernel_spmd(
        nc,
        in_maps,
        core_ids=list(range(NCORES)),
        trace=bool(int(os.environ.get("KERNEL_TRACE", "0"))),
    )
    out = np.concatenate(
        [res.results[c]["out"].reshape(BS, 1) for c in range(NCORES)], axis=0
    )
    # stash perf info for the test harness
    kernel.last_exec_time_ns = res.exec_time_ns
    kernel.last_trace = res.instructions_and_trace
    return out.astype(np.float32)
